# revision 19
# baseline (speedup 1.0000x reference)
"""Multi-head self-attention (CMHSAttn) Trainium2 kernel.

Problem: x (1, 128, 64, 64) fp32, W_qkv (384, 128) fp32.
  qkv = 1x1-conv(x, W_qkv); per head h (8 heads, d_head=16):
  q,k,v from qkv channels [48h:48h+16], [48h+16:48h+32], [48h+32:48h+48];
  out = softmax(q k^T / sqrt(128)) v, laid out channel-major (128, 64, 64).

Sharding: one head per NeuronCore (8 cores), pure data parallel, no
collectives. Each core receives the full x (bf16, channel-major (128, 4096))
plus its head's weight slices, and computes its 16 output channels.

Per-core algorithm (all matmuls bf16 with fp32 PSUM accumulation):
  - QT/KT = W_{q,k} @ x, computed replicated at partition offsets 0/32/64
    so score matmuls can be packed 3-wide into PE row groups (K=16 only).
  - V2 (128, 48*32): per 128-position chunk kj, cols [48kj:48kj+16] = 1.0
    (softmax-denominator rows), [+16:+32] zeros, [+32:+48] = V chunk
    (position-major; zero padding keeps partition bases 32-aligned).
  - For each q-chunk (512) and k-group (3 k-tiles of 128): S^T tiles
    (k-partition, q-free) via packed matmuls -> exp over (128, 1536) PSUM
    -> P bf16 -> matmuls accumulate O'' (48, 512) += V2_kj^T @ P_kj.
    O'' rows 0:16 = softmax denominator, rows 32:48 = unnormalized out^T.
  - out^T = O''[32:48] * reciprocal_approx(O''[0:16]); DMA to HBM.

exp is the throughput bottleneck (16.8M elements/core at 1 elem/cycle/lane
on the 1.2 GHz scalar engine = 109 us floor), so it is SPLIT between two
engines: most k-groups use ACT exp, and a tunable subset use a one-
instruction DVE Schraudolph approximation instead:
  bf16_bits(exp(s*SCALE)) ~= int16(s * (128*SCALE*log2e) + 128*(127-c))
(tensor_scalar mult+add with int16 convert, written into the bf16 P tile
via bitcast; measured rel err +-3.3%, rms 1.8%, which softmax
normalization largely cancels -- output absmax err stays ~1e-3 vs the
4e-3 budget). The two engines then exp different k-groups concurrently.
"""

import math

import ml_dtypes
import numpy as np

D_MODEL = 128
N = 4096  # 64*64 positions
DH = 16  # head dim
NH = 8  # heads = cores
QC = 512  # q-chunk (one PSUM bank of fp32)
NQC = N // QC  # 8
KT = 128  # k positions per score tile
NKJ = N // KT  # 32
# k-tile groups: 3-wide (PE row groups 0/32/64) except the last
GROUPS = [(g * 3, min(3, NKJ - g * 3)) for g in range((NKJ + 2) // 3)]
SCALE = 1.0 / math.sqrt(D_MODEL)
LOG2E = math.log2(math.e)
# Schraudolph constants: bf16 bits of exp(s*SCALE) ~= s*SEXP_A + SEXP_B
SEXP_A = 128.0 * SCALE * LOG2E
SEXP_B = 128.0 * (127.0 - 0.0430)
# which of the 11 k-groups per q-chunk the DVE exps (rest go to ACT);
# group 0 is the small (2-tile) group, 1..10 are full 3-tile groups
DVE_GROUP_IDX = (2, 4, 7, 9)

_NC_CACHE = {}


def _build_nc(legalize=True, loop_reps=None, pb_bufs=3, pipelined=True,
              dve_idx=DVE_GROUP_IDX, weave=False, skip_exp=False,
              fp8_ov=False):
    """Build the per-core Bass program. loop_reps wraps the whole body in a
    device-side For loop (used only for timing measurements)."""
    import concourse.bass as bass
    import concourse.mybir as mybir
    from concourse.tile import TileContext

    fp32 = mybir.dt.float32
    bf16 = mybir.dt.bfloat16
    i16 = mybir.dt.int16
    fp8 = mybir.dt.float8e4
    DR = mybir.MatmulPerfMode.DoubleRow
    EXP = mybir.ActivationFunctionType.Exp
    MULT = mybir.AluOpType.mult
    ADD = mybir.AluOpType.add

    nc = bass.Bass(name="cmhs_attn_head")
    xb = nc.dram_tensor("xb", [D_MODEL, N], bf16, kind="ExternalInput")
    wq = nc.dram_tensor("wq", [D_MODEL, 128], bf16, kind="ExternalInput")
    wk = nc.dram_tensor("wk", [D_MODEL, 128], bf16, kind="ExternalInput")
    wv = nc.dram_tensor("wv", [D_MODEL, DH], bf16, kind="ExternalInput")
    out = nc.dram_tensor("out", [DH, N], fp32, kind="ExternalOutput")

    with (
        TileContext(nc) as tc,
        tc.tile_pool(name="const", bufs=1) as cpool,
        tc.tile_pool(name="pwork", bufs=pb_bufs) as ppool,
        tc.tile_pool(name="small", bufs=3) as mpool,
        tc.tile_pool(name="ps", bufs=2, space="PSUM") as pspool,
        tc.tile_pool(name="po", bufs=2, space="PSUM") as popool,
    ):
        if True:
            # ---- persistent SBUF tensors ----
            xb_sb = cpool.tile([D_MODEL, N], bf16, name="xb_sb")
            wq_sb = cpool.tile([D_MODEL, 128], bf16, name="wq_sb")
            wk_sb = cpool.tile([D_MODEL, 128], bf16, name="wk_sb")
            wv_sb = cpool.tile([D_MODEL, DH], bf16, name="wv_sb")
            qt = cpool.tile([D_MODEL, N], bf16, name="qt")  # replicated q^T
            kt = cpool.tile([D_MODEL, N], bf16, name="kt")  # replicated k^T
            # per k-chunk 48 cols: ones (0:16) | zeros (16:32) | V (32:48)
            # (ones first: the softmax-denominator rows then land at o2
            # partitions 0:16, where the custom-DVE reciprocal -- which
            # ignores nonzero AP partition bases -- can read them)
            v2 = cpool.tile([D_MODEL, NKJ * 48], bf16, name="v2")

            v2_v = v2.rearrange("p (j t) -> p j t", t=48)
            if fp8_ov:
                # fp8 copy of v2 for DoubleRow attention@V on ACT groups
                v2_8 = cpool.tile([D_MODEL, NKJ * 48], fp8, name="v2_8")
                v2_8v = v2_8.rearrange("p (j t) -> p j t", t=48)

            def proj_qk_group(dst, w_sb, c0, cn):
                # project q or k (replicated at partitions 0-15/32-47/64-79)
                # for x-chunks c0..c0+cn
                pj = pspool.tile([D_MODEL, 3 * QC], fp32, name="pj", tag="s")
                for t in range(cn):
                    c = c0 + t
                    nc.tensor.matmul(
                        pj[:, t * QC : (t + 1) * QC],
                        lhsT=w_sb[:],
                        rhs=xb_sb[:, c * QC : (c + 1) * QC],
                        start=True,
                        stop=True,
                    )
                nc.vector.tensor_copy(
                    out=dst[:, c0 * QC : (c0 + cn) * QC],
                    in_=pj[:, : cn * QC],
                )

            def proj_v_range(vp, kj0, kj1):
                # V chunks kj0..kj1 position-major via x-chunk-stationary MMs
                vp_v = vp.rearrange("p (j t) -> p j t", t=DH)
                for kj in range(kj0, kj1):
                    nc.tensor.matmul(
                        vp[:, kj * DH : (kj + 1) * DH],
                        lhsT=xb_sb[:, kj * KT : (kj + 1) * KT],
                        rhs=wv_sb[:],
                        start=True,
                        stop=True,
                    )
                nc.vector.tensor_copy(
                    out=v2_v[:, kj0:kj1, 32:48],
                    in_=vp_v[:, kj0:kj1, :],
                )
                if fp8_ov:
                    nc.vector.tensor_copy(
                        out=v2_8v[:, kj0:kj1, 32:48],
                        in_=vp_v[:, kj0:kj1, :],
                    )

            def score_exp_group(qc, c0, cn, use_dve):
                # packed score matmuls + one exp over the group's PSUM span
                qs = qc * QC
                sps = pspool.tile([D_MODEL, 3 * QC], fp32, name="sps", tag="s")
                for t in range(cn):
                    kj = c0 + t
                    ro = 32 * t  # PE row group offset
                    nc.tensor.matmul(
                        sps[:, t * QC : (t + 1) * QC],
                        lhsT=kt[ro : ro + DH, kj * KT : (kj + 1) * KT],
                        rhs=qt[ro : ro + DH, qs : qs + QC],
                        start=True,
                        stop=True,
                    )
                act_fp8 = fp8_ov and not use_dve
                pb = ppool.tile(
                    [D_MODEL, 3 * QC],
                    fp8 if act_fp8 else bf16,
                    name="pb8" if act_fp8 else "pb",
                    tag="p8" if act_fp8 else "p",
                )
                if skip_exp:
                    # timing probe only: PE free-runs with no exp dependency
                    # (sliver write keeps Tile's allocator satisfied)
                    nc.vector.memset(pb[0:1, 0:32], 1.0)
                elif use_dve:
                    # DVE Schraudolph exp: int16 bits written into the bf16
                    # tile; softmax normalization absorbs the ~2% rms error
                    nc.vector.tensor_scalar(
                        pb[:, : cn * QC].bitcast(i16),
                        sps[:, : cn * QC],
                        SEXP_A,
                        SEXP_B,
                        MULT,
                        ADD,
                    )
                else:
                    nc.scalar.activation(
                        pb[:, : cn * QC], sps[:, : cn * QC], EXP, scale=SCALE
                    )
                return pb

            def ov_group(o2, pb, c0, cn, first, last):
                if fp8_ov and pb.dtype == fp8:
                    # fp8 DoubleRow: one matmul contracts 2 k-tiles (256
                    # positions) at 2 rows/cycle -- halves the PE time of
                    # the covered pairs
                    pb_r = pb.rearrange("p (j q) -> p j q", q=QC)
                    nc.tensor.matmul(
                        o2[:],
                        lhsT=v2_8v[:, c0 : c0 + 2, :],
                        rhs=pb_r[:, 0:2, :],
                        start=first,
                        stop=(last and cn == 2),
                        perf_mode=DR,
                        skip_group_check=True,
                    )
                    if cn == 3:
                        kj = c0 + 2
                        nc.tensor.matmul(
                            o2[:],
                            lhsT=v2_8[:, kj * 48 : kj * 48 + 48],
                            rhs=pb[:, 2 * QC : 3 * QC],
                            start=False,
                            stop=last,
                            skip_group_check=True,
                        )
                    return
                for t in range(cn):
                    kj = c0 + t
                    nc.tensor.matmul(
                        o2[:],
                        lhsT=v2[:, kj * 48 : kj * 48 + 48],
                        rhs=pb[:, t * QC : (t + 1) * QC],
                        start=(first and t == 0),
                        stop=(last and t == cn - 1),
                        skip_group_check=True,
                    )

            def normalize_and_store(qc, o2):
                # rows 32:48 of o2 all hold sum_k exp; approx reciprocal is
                # ~5x faster on DVE than the iterative divide (~18 good bits,
                # far beyond what the bf16 pipeline needs)
                rcp = mpool.tile([DH, QC], fp32, name="rcp", tag="rcp")
                nc.vector.reciprocal_approx_fast(rcp[:], o2[0:DH, :])
                ob = mpool.tile([DH, QC], fp32, name="ob", tag="ob")
                nc.vector.tensor_mul(ob[:], o2[32:48, :], rcp[:])
                nc.sync.dma_start(out=out[:, qc * QC : (qc + 1) * QC], in_=ob[:])

            def run_qc(qc, groups, interleave=None):
                # one q-chunk of attention. groups is the k-group order;
                # interleave maps group-index -> list of thunks to emit
                # after that group's score+exp (head-phase projection work).
                o2 = popool.tile([48, QC], fp32, name="o2", tag="o")
                # emit each group's score matmuls BEFORE the previous
                # group's attention@V matmuls: the in-order PE then
                # issues the score work the scalar engine needs next
                # without stalling on exp(g)
                pending = None
                for gi, (c0, cn) in enumerate(groups):
                    pb = score_exp_group(qc, c0, cn, gi in dve_idx)
                    if pending is not None:
                        ov_group(o2, *pending)
                    if interleave and gi in interleave:
                        for thunk in interleave[gi]:
                            thunk()
                    pending = (pb, c0, cn, gi == 0, gi == len(groups) - 1)
                ov_group(o2, *pending)
                normalize_and_store(qc, o2)

            def body():
                # constant regions of v2 first: no data deps, runs at t=0
                nc.vector.memset(v2_v[:, :, 0:DH], 1.0)
                nc.vector.memset(v2_v[:, :, DH:32], 0.0)
                if fp8_ov:
                    nc.vector.memset(v2_8v[:, :, 0:DH], 1.0)
                    nc.vector.memset(v2_8v[:, :, DH:32], 0.0)

                nc.sync.dma_start(out=wq_sb[:], in_=wq[:])
                nc.sync.dma_start(out=wk_sb[:], in_=wk[:])
                nc.sync.dma_start(out=wv_sb[:], in_=wv[:])
                # x in quarters so k/q projection starts after the first 256KB
                NQ4 = N // 4
                for i in range(4):
                    nc.sync.dma_start(
                        out=xb_sb[:, i * NQ4 : (i + 1) * NQ4],
                        in_=xb[:, i * NQ4 : (i + 1) * NQ4],
                    )

                # Warm the ACT exp table (~2.7us load) immediately at t=0:
                # seed a tiny tile with DVE so the table DMA doesn't wait for
                # the weight DMA to land first.
                warm = mpool.tile([1, 32], bf16, name="warm", tag="warm")
                nc.vector.memset(warm[:], 0.25)
                nc.scalar.activation(warm[:], warm[:], EXP, scale=SCALE)

                # Projection ordered so early k-tiles/q-chunk-0 data is
                # ready first (x arrives in quarters). weave=True would
                # interleave the tail of this projection into q-chunk 0's
                # attention groups; measured SLOWER (the proj tiles break
                # the 2-slot score-PSUM rotation that keeps both exp
                # engines saturated), so weave defaults to False.
                def vp_range(kj0, kj1):
                    def thunk():
                        vp = pspool.tile([D_MODEL, QC], fp32, name="vp", tag="s")
                        proj_v_range(vp, kj0, kj1)

                    return thunk

                proj_qk_group(kt, wk_sb, 0, 2)
                proj_qk_group(qt, wq_sb, 0, 2)
                vp_range(0, 8)()
                # q-chunk 0 runs k-groups in ascending order (small group
                # last) so early groups only need early k-tiles
                interleave = {
                    0: [lambda: proj_qk_group(kt, wk_sb, 2, 2), vp_range(8, 16)],
                    2: [lambda: proj_qk_group(kt, wk_sb, 4, 2), vp_range(16, 24)],
                    4: [lambda: proj_qk_group(kt, wk_sb, 6, 2), vp_range(24, 32)],
                    6: [lambda: proj_qk_group(qt, wq_sb, 2, 3)],
                    8: [lambda: proj_qk_group(qt, wq_sb, 5, 3)],
                }
                if not weave:
                    for gi in sorted(interleave):
                        for thunk in interleave[gi]:
                            thunk()
                    interleave = None
                run_qc(0, GROUPS, interleave)

                # small group first: the first exp of each q-chunk fires
                # after only 2 score matmuls, keeping the exp engines busy
                # across the q-chunk boundary
                groups = [GROUPS[-1]] + GROUPS[:-1]
                for qc in range(1, NQC):
                    run_qc(qc, groups)

            if loop_reps is None:
                body()
            else:
                with tc.For_i(0, loop_reps, 1):
                    body()

    # populate .instr bytes for the custom-DVE InstISA (reciprocal_approx);
    # raw Bass skips this codegen pass -> "ISA wrong length" in neuronxcc
    from concourse.library_overlay import lower_extended_insts

    lower_extended_insts(nc)

    if legalize:
        # note: the inserted EventSemaphores are invisible to CoreSim's race
        # detector; build with legalize=False when simulating
        _legalize_pe_waits(nc)
    return nc


def _legalize_pe_waits(nc):
    """Several HW-decoded engine instruction formats (MM, AC, ...) have a
    single sync-wait slot, but Tile occasionally attaches 2-3 waits at
    slot-reuse boundaries. Hoist the extras onto EventSemaphore instructions
    (one wait each) on the same engine queue right before the instruction —
    the same mechanism as a standalone wait_ge."""
    import concourse.mybir as mybir

    skip = {"EventSemaphore", "Call"}
    n = 0
    for blk in nc.m.functions[0].blocks:
        insts = blk.instructions
        out = []
        changed = False
        for inst in insts:
            si = getattr(inst, "sync_info", None)
            if (
                inst.opcode not in skip
                and si is not None
                and si.on_wait
                and len(si.on_wait) > 1
            ):
                waits = list(si.on_wait)
                for w in waits[:-1]:
                    ev = mybir.InstEventSemaphore(
                        name=f"hoistwait_{inst.name}_{n}", ins=[], outs=[]
                    )
                    n += 1
                    ev.engine = inst.engine
                    ev.sync_info = mybir.SyncInfo(on_wait=[w], on_update=[])
                    out.append(ev)
                si.on_wait = [waits[-1]]
                changed = True
            out.append(inst)
        if changed:
            blk.instructions = out


def _get_nc():
    if "nc" not in _NC_CACHE:
        _NC_CACHE["nc"] = _build_nc()
    return _NC_CACHE["nc"]


def make_in_maps(x, W_qkv):
    """Host-side sharding: per-head input maps for the 8 cores."""
    bf16 = ml_dtypes.bfloat16
    x = np.asarray(x, dtype=np.float32).reshape(D_MODEL, N)
    W = np.asarray(W_qkv, dtype=np.float32)
    xb = np.ascontiguousarray(x.astype(bf16))
    in_maps = []
    for h in range(NH):
        Wq = W[48 * h : 48 * h + 16]
        Wk = W[48 * h + 16 : 48 * h + 32]
        Wv = W[48 * h + 32 : 48 * h + 48]
        wq_rep = np.zeros((D_MODEL, 128), dtype=bf16)
        wk_rep = np.zeros((D_MODEL, 128), dtype=bf16)
        for i in range(3):
            wq_rep[:, 32 * i : 32 * i + 16] = Wq.T.astype(bf16)
            wk_rep[:, 32 * i : 32 * i + 16] = Wk.T.astype(bf16)
        in_maps.append(
            {
                "xb": xb,
                "wq": wq_rep,
                "wk": wk_rep,
                "wv": np.ascontiguousarray(Wv.T.astype(bf16)),
            }
        )
    return in_maps


def run_spmd(x, W_qkv, **kwargs):
    """Compile + run on 8 cores; returns BassKernelResults."""
    from concourse.bass_utils import run_bass_kernel_spmd

    nc = _get_nc()
    in_maps = make_in_maps(x, W_qkv)
    return run_bass_kernel_spmd(nc, in_maps, core_ids=list(range(NH)), **kwargs)


def kernel(x, W_qkv):
    res = run_spmd(x, W_qkv)
    outs = [res.results[h]["out"] for h in range(NH)]  # each (16, 4096) fp32
    full = np.concatenate(outs, axis=0)  # (128, 4096)
    return np.ascontiguousarray(full.reshape(1, D_MODEL, 64, 64), dtype=np.float32)

